# revision 3
# baseline (speedup 1.0000x reference)
"""Causal self-attention Bass kernel for 8x Trainium2 NeuronCores.

Problem: B=8, T=1024, D=1024, H=16 heads (head_dim 64), fp32.
Sharding: data parallel over batch -- each of the 8 cores handles one
batch element with replicated weights; outputs are stacked on the host.

Per-core dataflow (all matmuls on PE, fp32 accumulate in PSUM):
  1. x [T,D] is loaded and transposed on PE (128x128 blocks) to xT [D,T].
  2. qkvT = w_qkv^T @ x^T for the q,k features ([2048,T], kept transposed,
     bf16) and v = x @ w_qkv[:,2048:] in natural layout [T,1024] (bf16),
     both via fp32r matmuls with the bias folded in (per-partition DVE add
     for q/k, a K=1 ones-row matmul for v).
  3. Per head h and tq-block of 512: scoresT[tk,tq] = kT^T @ qT (K=64,
     bf16), exp on ACT (scale=1/8 folded, no max-subtraction -- scores are
     O(1) here so exp cannot overflow), causal handling by computing only
     the unmasked column window of each [128,512] tile plus one [128,128]
     triangular mask multiply on the diagonal block, then
     o_aug[65,tq] += v_aug^T @ P with v_aug = [v | ones], so row 64
     accumulates the softmax denominator for free.
  4. attn^T[d,tq] = o_aug[0:64] * (1/denom) with the reciprocal row
     broadcast across partitions via a step-0 SBUF->SBUF DMA.
  5. y = attn^T' @ w_proj + b_proj via fp32r matmuls (K=1 ones-row matmul
     adds the bias), streamed back to DRAM.
"""

import numpy as np
from contextlib import ExitStack

import concourse.bass as bass
import concourse.bacc as bacc
import concourse.tile as tile
import concourse.mybir as mybir
from concourse import bass_utils

F32 = mybir.dt.float32
F32R = mybir.dt.float32r
BF16 = mybir.dt.bfloat16
AF = mybir.ActivationFunctionType
OP = mybir.AluOpType

B, T, D, H, HD = 8, 1024, 1024, 16, 64
P = 128
N_CORES = 8

# Toggles (flip for experiments from test harnesses).
TRACE = False
USE_F32R = True

_CACHE = {}
LAST_RESULT = {}


def _r(ap):
    """Matmul operands are already fp32r-typed; kept as a hook point."""
    return ap


def _build_tile_kernel(nc, aps):
    x, wq, bq, wp, bp, ident, tri, ones, out = (
        aps["x"], aps["w_qkv"], aps["b_qkv"], aps["w_proj"], aps["b_proj"],
        aps["ident"], aps["tri"], aps["ones"], aps["out"],
    )

    with tile.TileContext(nc) as tc, ExitStack() as ctx:
        consts = ctx.enter_context(tc.tile_pool(name="consts", bufs=1))
        qk_pool = ctx.enter_context(tc.tile_pool(name="qk_pool", bufs=16))
        xt_pool = ctx.enter_context(tc.tile_pool(name="xt_pool", bufs=16))
        v_pool = ctx.enter_context(tc.tile_pool(name="v_pool", bufs=8))
        w_pool = ctx.enter_context(tc.tile_pool(name="w_pool", bufs=16))
        xn_pool = ctx.enter_context(tc.tile_pool(name="xn_pool", bufs=5))
        at_pool = ctx.enter_context(tc.tile_pool(name="at_pool", bufs=8))
        p_pool = ctx.enter_context(tc.tile_pool(name="p_pool", bufs=4))
        nrm_pool = ctx.enter_context(tc.tile_pool(name="nrm_pool", bufs=3))
        row_pool = ctx.enter_context(tc.tile_pool(name="row_pool", bufs=4))
        y_pool = ctx.enter_context(tc.tile_pool(name="y_pool", bufs=3))
        ps = ctx.enter_context(tc.tile_pool(name="ps", bufs=4, space="PSUM"))
        ops = ctx.enter_context(tc.tile_pool(name="ops", bufs=2, space="PSUM"))

        # ---- constants -------------------------------------------------
        id_sb = consts.tile([P, P], F32)
        nc.sync.dma_start(out=id_sb, in_=ident)
        tri_sb = consts.tile([P, P], BF16)
        nc.sync.dma_start(out=tri_sb, in_=tri)
        ones_sb = consts.tile([1, P], F32R)
        nc.sync.dma_start(out=ones_sb, in_=ones)
        bcol_sb = consts.tile([P, 16], F32)  # b_qkv[0:2048] as per-partition cols
        nc.sync.dma_start(out=bcol_sb, in_=bq[0:2048].rearrange("(f p) -> p f", p=P).bitcast(F32))
        bv_sb = consts.tile([1, D], F32R)  # v bias as a row
        nc.sync.dma_start(out=bv_sb, in_=bq[2048:3072].rearrange("(a d) -> a d", a=1))
        bp_sb = consts.tile([1, D], F32R)
        nc.sync.dma_start(out=bp_sb, in_=bp.rearrange("(a d) -> a d", a=1))

        # ---- phase 1a: x -> xT (PE transpose of 128x128 blocks) --------
        xt_tiles = {}  # (k, jj) -> [128, 512] fp32, xT[k*128:(k+1)*128, jj*512:...]
        for jj in range(2):
            xns = []
            for tt in range(4):
                ti = jj * 4 + tt
                xn = xn_pool.tile([P, D], F32, name="xn", tag="xn")
                nc.sync.dma_start(out=xn, in_=x[ti * P:(ti + 1) * P, :])
                xns.append(xn)
            for k in range(8):
                pst = ps.tile([P, 512], F32, name="pst", tag="ps")
                for tt in range(4):
                    nc.tensor.transpose(
                        pst[:, tt * P:(tt + 1) * P],
                        xns[tt][:, k * P:(k + 1) * P],
                        id_sb,
                    )
                xt_t = xt_pool.tile([P, 512], F32R, name="xt_t", tag="xt")
                nc.vector.tensor_copy(xt_t, pst)
                xt_tiles[(k, jj)] = xt_t

        # ---- phase 1b: qkT = (w_qkv[:, :2048])^T @ x^T, bf16 ----------
        qk_tiles = {}  # f-tile index 0..15 -> [128, 1024] bf16
        for f4 in range(4):
            wts = []
            for k in range(8):
                wt = w_pool.tile([P, 512], F32R, name="wt", tag="w")
                nc.sync.dma_start(
                    out=wt, in_=wq[k * P:(k + 1) * P, f4 * 512:(f4 + 1) * 512]
                )
                wts.append(wt)
            for fi in range(4):
                f = f4 * 4 + fi
                qk_t = qk_pool.tile([P, T], BF16, name="qk_t", tag="qk")
                qk_tiles[f] = qk_t
                for j in range(2):
                    acc = ps.tile([P, 512], F32, name="acc", tag="ps")
                    for k in range(8):
                        nc.tensor.matmul(
                            acc,
                            _r(wts[k][:, fi * P:(fi + 1) * P]),
                            _r(xt_tiles[(k, j)]),
                            start=(k == 0),
                            stop=(k == 7),
                        )
                    nc.vector.tensor_scalar_add(
                        qk_t[:, j * 512:(j + 1) * 512], acc, bcol_sb[:, f:f + 1]
                    )

        # ---- phase 1b': v natural layout with interleaved ones col -----
        # v_tiles[m] is [128, 16*65] bf16: per head 64 v cols + a ones col.
        v_tiles = []
        for m in range(8):
            vt = v_pool.tile([P, 16 * 65], BF16, name="vt", tag="v")
            nc.vector.memset(
                vt.rearrange("p (h c) -> p h c", c=65)[:, :, 64:65], 1.0
            )
            v_tiles.append(vt)
        for f4 in (4, 5):
            wts = []
            for k in range(8):
                wt = w_pool.tile([P, 512], F32R, name="wt", tag="w")
                nc.sync.dma_start(
                    out=wt, in_=wq[k * P:(k + 1) * P, f4 * 512:(f4 + 1) * 512]
                )
                wts.append(wt)
            for m in range(8):
                acc = ps.tile([P, 512], F32, name="acc", tag="ps")
                for k in range(8):
                    nc.tensor.matmul(
                        acc,
                        _r(xt_tiles[(k, m // 4)][:, (m % 4) * P:(m % 4 + 1) * P]),
                        _r(wts[k]),
                        start=(k == 0),
                        stop=False,
                    )
                nc.tensor.matmul(
                    acc,
                    _r(ones_sb),
                    _r(bv_sb[:, (f4 - 4) * 512:(f4 - 3) * 512]),
                    start=False,
                    stop=True,
                )
                dst = v_tiles[m].rearrange("p (h c) -> p h c", c=65)[
                    :, (f4 - 4) * 8:(f4 - 4) * 8 + 8, 0:64
                ]
                nc.vector.tensor_copy(dst, acc)

        # ---- phase 2 + 3: attention per tq-block, then its projection --
        wp_tiles = {}
        for c in range(8):
            for n in range(2):
                wpt = w_pool.tile([P, 512], F32R, name="wpt", tag="w")
                nc.sync.dma_start(
                    out=wpt, in_=wp[c * P:(c + 1) * P, n * 512:(n + 1) * 512]
                )
                wp_tiles[(c, n)] = wpt

        att_tiles = {}
        for j in range(2):
            for hp in range(8):
                at = at_pool.tile([P, 512], F32R, name="at", tag="at")
                att_tiles[(hp, j)] = at
                for hh in range(2):
                    h = hp * 2 + hh
                    fq = h // 2
                    po = (h % 2) * 64
                    qT = qk_tiles[fq][po:po + 64, j * 512:(j + 1) * 512]
                    o_ps = ops.tile([P, 512], F32, name="o_ps", tag="ops")
                    ni = 4 * j + 4
                    for i in range(ni):
                        m = i - 4 * j  # >= 0 on causal-partial tiles
                        ws = min(P * m, 256) if m >= 0 else 0
                        kT = qk_tiles[8 + fq][po:po + 64, i * P:(i + 1) * P]
                        s_ps = ps.tile([P, 512], F32, name="s_ps", tag="ps")
                        nc.tensor.matmul(
                            s_ps[:, ws:], kT, qT[:, ws:], start=True, stop=True
                        )
                        p_sb = p_pool.tile([P, 512], BF16, name="p_sb", tag="p")
                        nc.scalar.activation(
                            p_sb[:, ws:], s_ps[:, ws:], AF.Exp, scale=0.125
                        )
                        if m >= 0:
                            if m == 3:
                                nc.vector.memset(p_sb[:, 256:384], 0.0)
                            dc = P * m
                            nc.vector.tensor_tensor(
                                p_sb[:, dc:dc + P], p_sb[:, dc:dc + P],
                                tri_sb, op=OP.mult,
                            )
                        va = v_tiles[i].rearrange("p (h c) -> p h c", c=65)[:, h, :]
                        nc.tensor.matmul(
                            o_ps[0:65, ws:], va, p_sb[:, ws:],
                            start=(i == 0), stop=(i == ni - 1),
                        )
                    # normalize: row 64 of o_ps is the softmax denominator
                    r_sb = row_pool.tile([1, 512], F32, name="r_sb", tag="r")
                    nc.vector.reciprocal(r_sb, o_ps[64:65, :])
                    rb_sb = nrm_pool.tile([64, 512], F32, name="rb_sb", tag="rb")
                    nc.gpsimd.partition_broadcast(rb_sb, r_sb)
                    nc.vector.tensor_tensor(
                        att_tiles[(hp, j)][hh * 64:(hh + 1) * 64, :],
                        o_ps[0:64, :], rb_sb, op=OP.mult,
                    )

            # projection for this tq-block's rows
            for mi in range(4):
                mrow = 4 * j + mi
                for n in range(2):
                    y_ps = ps.tile([P, 512], F32, name="y_ps", tag="ps")
                    for c in range(8):
                        nc.tensor.matmul(
                            y_ps,
                            _r(att_tiles[(c, j)][:, mi * P:(mi + 1) * P]),
                            _r(wp_tiles[(c, n)]),
                            start=(c == 0),
                            stop=False,
                        )
                    nc.tensor.matmul(
                        y_ps, _r(ones_sb), _r(bp_sb[:, n * 512:(n + 1) * 512]),
                        start=False, stop=True,
                    )
                    y_sb = y_pool.tile([P, 512], F32, name="y_sb", tag="y")
                    nc.scalar.copy(y_sb, y_ps)
                    nc.sync.dma_start(
                        out=out[mrow * P:(mrow + 1) * P, n * 512:(n + 1) * 512],
                        in_=y_sb,
                    )


def _get_nc():
    if "nc" in _CACHE:
        return _CACHE["nc"]
    nc = bacc.Bacc("TRN2", target_bir_lowering=False, debug=False,
                   num_devices=N_CORES)
    aps = {
        "x": nc.dram_tensor("x", [T, D], F32, kind="ExternalInput").ap(),
        "w_qkv": nc.dram_tensor("w_qkv", [D, 3 * D], F32R, kind="ExternalInput").ap(),
        "b_qkv": nc.dram_tensor("b_qkv", [3 * D], F32R, kind="ExternalInput").ap(),
        "w_proj": nc.dram_tensor("w_proj", [D, D], F32R, kind="ExternalInput").ap(),
        "b_proj": nc.dram_tensor("b_proj", [D], F32R, kind="ExternalInput").ap(),
        "ident": nc.dram_tensor("ident", [P, P], F32, kind="ExternalInput").ap(),
        "tri": nc.dram_tensor("tri", [P, P], BF16, kind="ExternalInput").ap(),
        "ones": nc.dram_tensor("ones", [1, P], F32R, kind="ExternalInput").ap(),
        "out": nc.dram_tensor("out", [T, D], F32, kind="ExternalOutput").ap(),
    }
    _build_tile_kernel(nc, aps)
    nc.compile()
    _CACHE["nc"] = nc
    return nc


def _host_consts():
    import ml_dtypes
    ident = np.eye(P, dtype=np.float32)
    r = np.arange(P)
    tri = (r[:, None] <= r[None, :]).astype(ml_dtypes.bfloat16)
    ones = np.ones((1, P), dtype=np.float32)
    return ident, tri, ones


def kernel(x, w_qkv, b_qkv, w_proj, b_proj):
    x = np.ascontiguousarray(np.asarray(x, dtype=np.float32))
    w_qkv = np.ascontiguousarray(np.asarray(w_qkv, dtype=np.float32))
    b_qkv = np.ascontiguousarray(np.asarray(b_qkv, dtype=np.float32))
    w_proj = np.ascontiguousarray(np.asarray(w_proj, dtype=np.float32))
    b_proj = np.ascontiguousarray(np.asarray(b_proj, dtype=np.float32))

    nc = _get_nc()
    ident, tri, ones = _host_consts()
    in_maps = [
        {
            "x": x[b],
            "w_qkv": w_qkv,
            "b_qkv": b_qkv,
            "w_proj": w_proj,
            "b_proj": b_proj,
            "ident": ident,
            "tri": tri,
            "ones": ones,
        }
        for b in range(N_CORES)
    ]
    res = bass_utils.run_bass_kernel_spmd(
        nc, in_maps, core_ids=list(range(N_CORES)), trace=TRACE
    )
    LAST_RESULT["res"] = res
    return np.stack([res.results[c]["out"] for c in range(N_CORES)]).astype(
        np.float32
    )
